# revision 1
# baseline (speedup 1.0000x reference)
"""Trainium2 Bass kernel for HalfHadamardTrustQuantizer.

Computation (forward value of the reference, which collapses to xq):
  x_had = blockwise-64 Hadamard rotation of channels:  (B,C,H,W), C=512 = 8 groups of 64
  std   = sqrt(mean(x_had^2)) per sample  (== RMS of x by orthogonality)
  scale = OPT*std + 1e-8 ; step = 2*scale/15
  xq    = round(clip(x_had,-scale,scale)/step + 0.5)*step - step/2

Sharding: data-parallel over batch; 2 samples per core on 8 cores.

Per-core pipeline (2 samples of (512, 3136) fp32), all DMAs full-row
(12544B/partition descriptors keep all 16 DMA engines in flight; total
HBM traffic 25.7MB/core ~ 62us at the ~415GB/s aggregate DMA cap):
  loads: rows s0r0, s0r1, s1r3 via Pool SWDGE casting DMAs (fp32 -> fp32r
         in flight, using the Pool queue's idle window before stores);
         the other 5 rows fp32 on SP/ACT HWDGE queues + a DVE pass to
         fp32r (fp32 with 11-bit mantissa; the PE matmul runs 1 cycle/row
         on an fp32r moving operand vs 4 cycles/row for fp32)
  per row: Square+accum_out -> per-partition sumsq (ACT mostly; s0r2/r3
         and s1r3 on DVE to balance)
  scalars: PE ones-matmul cross-partition reduce+broadcast; ACT Sqrt
           (<=2 ULP); DVE ops for scale/step/inv
  phase B: PE fp32r matmul (N=512 chunks into 4-bank PSUM tiles);
           ACT Identity(scale=1/step, bias=0.5) PSUM -> int16 (RNE fused),
           2 drains/row; Pool int16 clip (min 8, max -7); DVE affine
           int16 -> f32 (*step - step/2, DVE 2x); full-row f32 stores on
           Pool (rows 0,1), SP (row 2), ACT (row 3).
"""

import numpy as np
from contextlib import ExitStack

B, C, HH, WW = 16, 512, 56, 56
R = HH * WW            # 3136 spatial
NCORES = 8
S = B // NCORES        # samples per core
NB = C // 128          # block-rows per sample
N_ELEM = C * R         # per-sample reduction size
OPT = 2.513930578568423
INV_N = np.float64(1.0) / np.float64(N_ELEM)   # cast at use
TWO_15 = np.float32(2.0) / np.float32(15.0)

_CACHE = {}


def _build_program():
    import concourse.bacc as bacc
    import concourse.tile as tile
    import concourse.mybir as mybir

    AF = mybir.ActivationFunctionType
    OP = mybir.AluOpType
    f32 = mybir.dt.float32
    f32r = mybir.dt.float32r
    i16 = mybir.dt.int16

    nc = bacc.Bacc("TRN2", target_bir_lowering=False, debug=False,
                   num_devices=NCORES)
    x = nc.dram_tensor("x", [S * C, R], f32, kind="ExternalInput").ap()
    w = nc.dram_tensor("w", [128, 128], f32, kind="ExternalInput").ap()
    y = nc.dram_tensor("y", [S * C, R], f32, kind="ExternalOutput").ap()

    # two PSUM drain tiles per row: [0,2048) and [2048,3136)
    TILES = [(0, 2048, [512, 512, 512, 512]), (2048, 1088, [512, 512, 64])]
    HALVES = [(0, 2048), (2048, 1088)]

    CAST = {(0, 0), (0, 1), (1, 3)}   # rows loaded via Pool casting DMA

    with tile.TileContext(nc) as tc, ExitStack() as ctx:
        xp = ctx.enter_context(tc.tile_pool(name="xp", bufs=3))
        xrp = ctx.enter_context(tc.tile_pool(name="xrp", bufs=8))
        cn = ctx.enter_context(tc.tile_pool(name="cn", bufs=1))
        sq = ctx.enter_context(tc.tile_pool(name="sq", bufs=1))
        ac = ctx.enter_context(tc.tile_pool(name="ac", bufs=2))
        sc = ctx.enter_context(tc.tile_pool(name="sc", bufs=2))
        ip = ctx.enter_context(tc.tile_pool(name="ip", bufs=2))
        op_ = ctx.enter_context(tc.tile_pool(name="op", bufs=2))
        pp = ctx.enter_context(tc.tile_pool(name="pp", bufs=2, space="PSUM"))

        wt = cn.tile([128, 128], f32r, tag="w")
        nc.gpsimd.dma_start(wt[:], w[:])
        ones = cn.tile([128, 128], f32, tag="ones")
        nc.gpsimd.memset(ones[:], 1.0)
        half = cn.tile([128, 1], f32, tag="half")
        nc.gpsimd.memset(half[:], 0.5)

        sq_scr = sq.tile([128, R], f32, tag="sqscr")

        xts = {}
        xrs = {}
        scal = {}

        def emit_load(s, b, ld_eng):
            r0 = x[s * C + b * 128:s * C + (b + 1) * 128, :]
            if (s, b) in CAST:
                xr = xrp.tile([128, R], f32r, tag="xr")
                nc.gpsimd.dma_start(xr[:], r0[:])
                xrs[(s, b)] = xr
            else:
                xt = xp.tile([128, R], f32, tag="xrow")
                ld_eng.dma_start(xt[:], r0[:])
                xts[(s, b)] = xt

        def emit_square(s, b, sq_eng):
            if (s, b) in CAST:
                src = xrs[(s, b)][:].bitcast(f32)
            else:
                src = xts[(s, b)][:]
            if sq_eng == "act":
                nc.scalar.activation(sq_scr[:], src, AF.Square,
                                     accum_out=parts[s][:, b:b + 1])
            else:
                nc.vector.scalar_tensor_tensor(sq_scr[:], src, 1.0, src,
                                               OP.mult, OP.mult,
                                               accum_out=parts[s][:, b:b + 1])

        def emit_convert(s, b):
            xt = xts.pop((s, b))
            xr = xrp.tile([128, R], f32r, tag="xr")
            nc.vector.tensor_scalar_mul(xr[:, 0:1568], xt[:, 0:1568], 1.0)
            nc.vector.tensor_scalar_mul(xr[:, 1568:R], xt[:, 1568:R], 1.0)
            xrs[(s, b)] = xr

        def sample_scalars(s):
            # ---- per-sample scalars (sqrt is <=2 ULP; no Newton step) ----
            part = parts[s]
            red = sc.tile([128, 1], f32, tag="red")
            nc.vector.reduce_sum(red[:], part[:], axis=mybir.AxisListType.X)
            tot = pp.tile([128, 2048], f32, tag="pchunk")
            tot = tot[:, 0:1]
            nc.tensor.matmul(tot[:], ones[:], red[:], start=True, stop=True)
            std = sc.tile([128, 1], f32, tag="std")
            nc.scalar.activation(std[:], tot[:], AF.Sqrt, scale=float(INV_N))
            scale_t = sc.tile([128, 1], f32, tag="scale")
            nc.vector.tensor_scalar(scale_t[:], std[:], float(OPT), 1e-8,
                                    OP.mult, OP.add)
            step = sc.tile([128, 1], f32, tag="step")
            nc.vector.tensor_scalar_mul(step[:], scale_t[:], float(TWO_15))
            inv = sc.tile([128, 1], f32, tag="inv")
            nc.vector.reciprocal(inv[:], step[:])
            hstep = sc.tile([128, 1], f32, tag="hstep")
            nc.vector.tensor_scalar_mul(hstep[:], step[:], 0.5)
            scal[s] = (inv, step, hstep)

        def phase_b_row(s, b, st_eng):
            inv, step, hstep = scal[s]
            xr = xrs.pop((s, b))
            # ---- phase B: rotate + quantize + store (one full-row DMA) ----
            irow = ip.tile([128, R], i16, tag="irow")
            for off, tw, chunks in TILES:
                pm = pp.tile([128, 2048], f32, tag="pchunk")
                co = 0
                for ch in chunks:
                    nc.tensor.matmul(pm[:, co:co + ch], wt[:],
                                     xr[:, off + co:off + co + ch],
                                     start=True, stop=True)
                    co += ch
                nc.scalar.activation(irow[:, off:off + tw],
                                     pm[:, :tw], AF.Identity,
                                     bias=half[:], scale=inv[:])
            orow = op_.tile([128, R], f32, tag="orow")
            for off, w_ in HALVES:
                nc.gpsimd.tensor_scalar(irow[:, off:off + w_],
                                        irow[:, off:off + w_], 8, -7,
                                        OP.min, OP.max)
                nc.vector.tensor_scalar(orow[:, off:off + w_],
                                        irow[:, off:off + w_],
                                        step[:], hstep[:],
                                        OP.mult, OP.subtract)
            st_eng.dma_start(
                y[s * C + b * 128:s * C + (b + 1) * 128, :], orow[:])

        # ---- pipelined emission (arrival-ordered; no FIFO head-of-line) --
        parts = {}
        for s in range(S):
            part_t = ac.tile([128, NB], f32, tag=f"part{s}", name=f"part{s}")
            parts[s] = part_t

        # loads: Pool FIFO [wt, s0r0, s0r1, s1r3, stores...];
        # SP: s0r2, s1r0, s1r2; ACT: s0r3, s1r1
        emit_load(0, 0, None)
        emit_load(0, 1, None)
        emit_load(0, 2, nc.sync)
        emit_load(0, 3, nc.scalar)
        emit_load(1, 0, nc.sync)
        emit_load(1, 1, nc.scalar)
        emit_load(1, 2, nc.sync)
        emit_load(1, 3, None)

        # s0 input-side work: all squares on ACT; converts (r2, r3) on DVE
        for b in range(NB):
            emit_square(0, b, "act")
            if (0, b) not in CAST:
                emit_convert(0, b)
        sample_scalars(0)
        # s1r3 square early on DVE (cast row, arrives ~27us, out of ACT FIFO)
        emit_square(1, 3, "dve")
        # middle: interleave s1 input-side work with s0 phase B
        for b in range(NB):
            if b < 3:
                emit_square(1, b, "act")
                emit_convert(1, b)
            if b == 3:
                sample_scalars(1)
            phase_b_row(0, b, [nc.gpsimd, nc.gpsimd, nc.sync, nc.scalar][b])
        for b in range(NB):
            phase_b_row(1, b, [nc.gpsimd, nc.gpsimd, nc.sync, nc.scalar][b])
    nc.compile()
    return nc


def _get_program():
    if "nc" not in _CACHE:
        _CACHE["nc"] = _build_program()
    return _CACHE["nc"]


def kernel(x: np.ndarray, aux_matrix: np.ndarray) -> np.ndarray:
    from concourse.bass_utils import run_bass_kernel_spmd

    x = np.ascontiguousarray(x, dtype=np.float32)
    aux = np.ascontiguousarray(aux_matrix, dtype=np.float32)
    w128 = np.zeros((128, 128), dtype=np.float32)
    w128[:64, :64] = aux
    w128[64:, 64:] = aux

    nc = _get_program()
    in_maps = [
        {"x": x[c * S:(c + 1) * S].reshape(S * C, R), "w": w128}
        for c in range(NCORES)
    ]
    res = run_bass_kernel_spmd(nc, in_maps, list(range(NCORES)))
    out = np.empty((B, C, HH, WW), dtype=np.float32)
    for c in range(NCORES):
        out[c * S:(c + 1) * S] = res.results[c]["y"].reshape(S, C, HH, WW)
    return out



# revision 2
# speedup vs baseline: 1.1723x; 1.1723x over previous
"""Trainium2 Bass kernel for HalfHadamardTrustQuantizer.

Computation (forward value of the reference, which collapses to xq):
  x_had = blockwise-64 Hadamard rotation of channels:  (B,C,H,W), C=512 = 8 groups of 64
  std   = sqrt(mean(x_had^2)) per sample  (== RMS of x by orthogonality)
  scale = OPT*std + 1e-8 ; step = 2*scale/15
  xq    = round(clip(x_had,-scale,scale)/step + 0.5)*step - step/2

Sharding: data-parallel over batch; 2 samples per core on 8 cores.

v2 pipeline (per core, 2 samples of (512, 3136) fp32):
  All loads on the Pool SWDGE queue (Q0) as casting DMAs f32 -> f32r, in
  half-rows split at col 2048 (matches PSUM drain tiling); sample 0 first
  so its stats close early.  All stores on the Sync HWDGE queue (Q10) as
  full rows.  The two queues stream concurrently; total HBM traffic
  25.7MB/core.
  Input side: ACT squares the 2048-halves, DVE squares the 1088-halves
  (each with accum_out into per-sample partial columns).  Scalars: DVE
  reduce + PE ones-matmul broadcast + ACT sqrt + DVE ops.
  Phase B per row: PE f32r matmuls into 2 PSUM tiles (2048 / 1088);
  ACT drains the 2048 tile (Identity, scale=1/step, bias=0.5 -> i16 RNE),
  DVE drains the 1088 tile (tensor_scalar mult+add -> i16 RNE);
  Pool clips i16 to [-7, 8]; DVE affine i16 -> f32 (*step - step/2);
  full-row store on Sync.  Squares of sample 1 interleave with sample 0's
  phase B at matching expected-arrival positions (per-engine FIFOs).
"""

import numpy as np
from contextlib import ExitStack

B, C, HH, WW = 16, 512, 56, 56
R = HH * WW            # 3136 spatial
NCORES = 8
S = B // NCORES        # samples per core
NB = C // 128          # block-rows per sample
N_ELEM = C * R         # per-sample reduction size
OPT = 2.513930578568423
INV_N = np.float64(1.0) / np.float64(N_ELEM)
TWO_15 = np.float32(2.0) / np.float32(15.0)

WA = 2048              # A-half width (cols 0:2048)
WB = R - WA            # B-half width (cols 2048:3136) = 1088

_CACHE = {}


def _build_program():
    import concourse.bacc as bacc
    import concourse.tile as tile
    import concourse.mybir as mybir

    AF = mybir.ActivationFunctionType
    OP = mybir.AluOpType
    f32 = mybir.dt.float32
    f32r = mybir.dt.float32r
    i16 = mybir.dt.int16

    nc = bacc.Bacc("TRN2", target_bir_lowering=False, debug=False,
                   num_devices=NCORES)
    x = nc.dram_tensor("x", [S * C, R], f32, kind="ExternalInput").ap()
    w = nc.dram_tensor("w", [128, 128], f32, kind="ExternalInput").ap()
    y = nc.dram_tensor("y", [S * C, R], f32, kind="ExternalOutput").ap()

    with tile.TileContext(nc) as tc, ExitStack() as ctx:
        xap = ctx.enter_context(tc.tile_pool(name="xap", bufs=2 * NB))
        xbp = ctx.enter_context(tc.tile_pool(name="xbp", bufs=2 * NB))
        cn = ctx.enter_context(tc.tile_pool(name="cn", bufs=1))
        sq = ctx.enter_context(tc.tile_pool(name="sq", bufs=1))
        ac = ctx.enter_context(tc.tile_pool(name="ac", bufs=2))
        sc = ctx.enter_context(tc.tile_pool(name="sc", bufs=2))
        iap = ctx.enter_context(tc.tile_pool(name="iap", bufs=3))
        ibp = ctx.enter_context(tc.tile_pool(name="ibp", bufs=3))
        op_ = ctx.enter_context(tc.tile_pool(name="op", bufs=3))
        pp = ctx.enter_context(tc.tile_pool(name="pp", bufs=2, space="PSUM"))

        # ---- loads: wt first, then all 16 casting half-loads on Q0 ----
        wt = cn.tile([128, 128], f32r, tag="w")
        nc.gpsimd.dma_start(wt[:], w[:])

        xas = {}
        xbs = {}
        for s in range(S):
            for b in range(NB):
                r0 = s * C + b * 128
                xa = xap.tile([128, WA], f32r, tag="xa")
                nc.gpsimd.dma_start(xa[:], x[r0:r0 + 128, 0:WA])
                xas[(s, b)] = xa
                xb = xbp.tile([128, WB], f32r, tag="xb")
                nc.gpsimd.dma_start(xb[:], x[r0:r0 + 128, WA:R])
                xbs[(s, b)] = xb

        ones = cn.tile([128, 128], f32, tag="ones")
        nc.gpsimd.memset(ones[:], 1.0)
        half = cn.tile([128, 1], f32, tag="half")
        nc.gpsimd.memset(half[:], 0.5)

        sqa = sq.tile([128, WA], f32, tag="sqa")   # ACT square scratch
        sqb = sq.tile([128, WB], f32, tag="sqb")   # DVE square scratch

        parts = {}
        for s in range(S):
            parts[s] = ac.tile([128, 2 * NB], f32, tag=f"part{s}",
                               name=f"part{s}")

        scal = {}

        def sq_a(s, b):
            nc.scalar.activation(sqa[:], xas[(s, b)][:].bitcast(f32),
                                 AF.Square,
                                 accum_out=parts[s][:, 2 * b:2 * b + 1])

        def sq_b(s, b):
            src = xbs[(s, b)][:].bitcast(f32)
            nc.vector.scalar_tensor_tensor(sqb[:], src, 1.0, src,
                                           OP.mult, OP.mult,
                                           accum_out=parts[s][:, 2 * b + 1:
                                                              2 * b + 2])

        def sample_scalars(s):
            red = sc.tile([128, 1], f32, tag="red")
            nc.vector.reduce_sum(red[:], parts[s][:], axis=mybir.AxisListType.X)
            tot = pp.tile([128, WA], f32, tag="pchunk")
            tot = tot[:, 0:1]
            nc.tensor.matmul(tot[:], ones[:], red[:], start=True, stop=True)
            std = sc.tile([128, 1], f32, tag="std")
            nc.scalar.activation(std[:], tot[:], AF.Sqrt, scale=float(INV_N))
            scale_t = sc.tile([128, 1], f32, tag="scale")
            nc.vector.tensor_scalar(scale_t[:], std[:], float(OPT), 1e-8,
                                    OP.mult, OP.add)
            step = sc.tile([128, 1], f32, tag="step")
            nc.vector.tensor_scalar_mul(step[:], scale_t[:], float(TWO_15))
            inv = sc.tile([128, 1], f32, tag="inv")
            nc.vector.reciprocal(inv[:], step[:])
            hstep = sc.tile([128, 1], f32, tag="hstep")
            nc.vector.tensor_scalar_mul(hstep[:], step[:], 0.5)
            scal[s] = (inv, step, hstep)

        def phase_b_row(s, b, next_sq=None):
            """Rotate+quantize+store one row; optionally interleave the
            squares of (next sample, row b) into the ACT/DVE FIFOs."""
            inv, step, hstep = scal[s]
            xa = xas.pop((s, b))
            xb = xbs.pop((s, b))
            # matmuls into two PSUM tiles
            pma = pp.tile([128, WA], f32, tag="pchunk")
            for co in range(0, WA, 512):
                nc.tensor.matmul(pma[:, co:co + 512], wt[:],
                                 xa[:, co:co + 512], start=True, stop=True)
            pmb = pp.tile([128, WA], f32, tag="pchunk")
            for co in range(0, WB, 512):
                ch = min(512, WB - co)
                nc.tensor.matmul(pmb[:, co:co + ch], wt[:],
                                 xb[:, co:co + ch], start=True, stop=True)
            # drains -> i16 (RNE)
            ia = iap.tile([128, WA], i16, tag="ia")
            nc.scalar.activation(ia[:], pma[:, :WA], AF.Identity,
                                 bias=half[:], scale=inv[:])
            if next_sq is not None:
                sq_a(*next_sq)
            ib = ibp.tile([128, WB], i16, tag="ib")
            nc.vector.tensor_scalar(ib[:], pmb[:, :WB], inv[:], half[:],
                                    OP.mult, OP.add)
            # clip to [-7, 8] (int domain: min 8 then max -7)
            nc.gpsimd.tensor_scalar(ia[:], ia[:], 8, -7, OP.min, OP.max)
            nc.gpsimd.tensor_scalar(ib[:], ib[:], 8, -7, OP.min, OP.max)
            # affine i16 -> f32: *step - step/2
            orow = op_.tile([128, R], f32, tag="orow")
            nc.vector.tensor_scalar(orow[:, 0:WA], ia[:], step[:], hstep[:],
                                    OP.mult, OP.subtract)
            nc.vector.tensor_scalar(orow[:, WA:R], ib[:], step[:], hstep[:],
                                    OP.mult, OP.subtract)
            if next_sq is not None:
                sq_b(*next_sq)
            nc.sync.dma_start(
                y[s * C + b * 128:s * C + (b + 1) * 128, :], orow[:])

        # ---- input side sample 0 ----
        for b in range(NB):
            sq_a(0, b)
            sq_b(0, b)
        sample_scalars(0)
        # ---- phase B s0, squares of s1 interleaved ----
        for b in range(NB):
            phase_b_row(0, b, next_sq=(1, b))
        sample_scalars(1)
        for b in range(NB):
            phase_b_row(1, b)
    nc.compile()
    return nc


def _get_program():
    if "nc" not in _CACHE:
        _CACHE["nc"] = _build_program()
    return _CACHE["nc"]


def kernel(x: np.ndarray, aux_matrix: np.ndarray) -> np.ndarray:
    from concourse.bass_utils import run_bass_kernel_spmd

    x = np.ascontiguousarray(x, dtype=np.float32)
    aux = np.ascontiguousarray(aux_matrix, dtype=np.float32)
    w128 = np.zeros((128, 128), dtype=np.float32)
    w128[:64, :64] = aux
    w128[64:, 64:] = aux

    nc = _get_program()
    in_maps = [
        {"x": x[c * S:(c + 1) * S].reshape(S * C, R), "w": w128}
        for c in range(NCORES)
    ]
    res = run_bass_kernel_spmd(nc, in_maps, list(range(NCORES)))
    out = np.empty((B, C, HH, WW), dtype=np.float32)
    for c in range(NCORES):
        out[c * S:(c + 1) * S] = res.results[c]["y"].reshape(S, C, HH, WW)
    return out
